# revision 10
# baseline (speedup 1.0000x reference)
"""Depth-masked 3-branch 3x3 conv (Conv2.5D) on 8 TRN2 NeuronCores.

Data-parallel over N=8 images (1 image/core). Per core, a polynomial
selector replaces per-branch masking: with z = r - 2 (r = nearest-int
bin of (d_col-center)/grid in 0..4), the field q = (z^2-4)(z-1/4) is
zero for inactive taps and takes distinct values {3.75, 1, -2.25} for
branches 0/1/2. The three masked GEMM inputs per tap-pair are the chain
  y1 = Q*x, y2 = Q*y1, y3 = Q*y2      (DVE tensor_tensor, 2x mode)
with host-remixed weights Wt_j = sum_b a_j(b) W_b. The center tap is
always branch-1 active and is fed unmasked. Q is replicated across the
64 channel partitions by DMA broadcast (stride-0 free dim) from a DRAM
bounce buffer, so no DVE shuffles and no per-branch compare ops remain.
"""

import sys

sys.path.insert(0, "/opt/trn_rl_repo")

import numpy as np
import ml_dtypes

import concourse.bass as bass
import concourse.mybir as mybir
from concourse.bass_utils import run_bass_kernel_spmd
from concourse import tile
from concourse.vector_clock import VectorClock, ScopedClock

F32 = mybir.dt.float32
BF16 = mybir.dt.bfloat16
AF = mybir.ActivationFunctionType
ALU = mybir.AluOpType

N_IMG, C, O, H, W = 8, 64, 64, 128, 128
L = H * W
BASE = 256  # pad on each side of the x tiles (reads span +-259)
XW = BASE + L + BASE
CH = 1024  # psum / output chunk
SUP = 2048  # mult / broadcast super-chunk
NSUP = L // SUP
NCH = L // CH
# tap k = 3*(dh+1)+(dw+1); flat pixel offset dh*W+dw
OFF = [(k // 3 - 1) * W + (k % 3 - 1) for k in range(9)]
# non-center taps grouped in pairs sharing one physical x tile
PAIRS = [(0, 1), (6, 7), (2, 5), (3, 8)]
# tile shift of rows 64:128 relative to rows 0:64, per pair tile
PAIR_TILE = [0, 0, 1, 2]  # index into (tA, tB, tC)
TILE_SHIFT = [1, 128, 130]
WINO = [OFF[k1] for k1, _ in PAIRS]  # window offset per pair
QSEL_C = 0.25


def _patched_drain_and_barrier(self, tick_clock, wait_clock):
    # stock version puts every live sem wait on one drain -> walrus
    # "Too many sync wait commands"; emit one single-wait NOP per sem.
    ticks = list(tick_clock.global_clock)
    n = len(ticks)
    for i, t in enumerate(ticks):
        if t > 0:
            vec = [0] * n
            vec[i] = t
            nop = self.nc.sync.nop()
            wait_clock.add_sem_waits(nop.ins, ScopedClock({None: VectorClock(vec)}))
    self.nc.sync.drain()
    self.nc.all_engine_barrier()
    popped = self.nc._tile_sem_poison_stack.pop()
    assert popped is self._sem_poison
    self.nc.clear_and_free_semaphores(list(self.sems.allocated().values()))
    self.nc.all_engine_barrier()


tile.TileContext._drain_and_barrier = _patched_drain_and_barrier


def _split_excess_waits(nc, noop_cls, max_waits=1):
    # this walrus build rejects >1 sync-wait on several instruction
    # structs; hoist extras onto same-engine NoOps placed just before.
    for fn in nc.m.functions:
        for blk in fn.blocks:
            idx = 0
            while idx < len(blk.instructions):
                inst = blk.instructions[idx]
                si = inst.sync_info
                if si is not None and len(si.on_wait) > max_waits:
                    waits = list(si.on_wait)
                    si.on_wait = waits[-max_waits:]
                    pos = idx
                    for w in waits[:-max_waits]:
                        nop = noop_cls(
                            name=nc.get_next_instruction_name(), ins=[], outs=[]
                        )
                        nop.engine = inst.engine
                        nop.sync_info = mybir.SyncInfo(on_wait=[w], on_update=[])
                        nc.register_instruction(nop)
                        blk.instructions.insert(pos, nop)
                        pos += 1
                        idx += 1
                idx += 1


def _build_graph():
    nc = bass.Bass()
    x_d = nc.declare_dram_parameter("x", [C, L], F32, isOutput=False)
    dep_d = nc.declare_dram_parameter("depth", [H, W], F32, isOutput=False)
    rfx_d = nc.declare_dram_parameter("rfx", [128, 1], F32, isOutput=False)
    wp_d = nc.declare_dram_parameter("wp", [128, 13 * 64], BF16, isOutput=False)
    out_d = nc.declare_dram_parameter("out", [O, L], F32, isOutput=True)
    qd = nc.dram_tensor("qd", (8, L), BF16, kind="Internal")

    with tile.TileContext(nc) as tc:
        with (
            tc.tile_pool(name="big", bufs=1) as big,
            tc.tile_pool(name="stage", bufs=3) as stage,
            tc.tile_pool(name="qp", bufs=1) as qp,
            tc.tile_pool(name="yp", bufs=2) as yp,
            tc.tile_pool(name="outp", bufs=3) as outp,
            tc.tile_pool(name="psum", bufs=4, space=bass.MemorySpace.PSUM) as psp,
        ):
            wp = big.tile([128, 13 * 64], BF16)
            nc.sync.dma_start(wp[:], wp_d[:])

            tiles = [big.tile([128, XW], BF16, name=f"t{i}") for i in range(3)]
            for t in tiles:
                nc.gpsimd.memset(t[:, 0:BASE], 0.0)
                nc.gpsimd.memset(t[:, BASE + L : XW], 0.0)
            tA = tiles[0]

            # ---- depth -> q selector field (128h x 9*128w) ----
            with tc.tile_pool(name="mk", bufs=1) as mk:
                dsh = mk.tile([128, 3 * 130], F32)
                nc.vector.memset(dsh[:], 0.0)
                nc.sync.dma_start(dsh[:, 131:259], dep_d[:, :])
                nc.sync.dma_start(dsh[0:127, 261:389], dep_d[1:128, :])
                nc.sync.dma_start(dsh[1:128, 1:129], dep_d[0:127, :])
                rfx = mk.tile([128, 1], F32)
                nc.sync.dma_start(rfx[:], rfx_d[:])

                g = mk.tile([128, 128], F32)
                nc.vector.tensor_scalar(g[:], dsh[:, 131:259], rfx[:], None, ALU.mult)
                rg = mk.tile([128, 128], F32)
                nc.vector.reciprocal(rg[:], g[:])

                def _win(base, offset, dims):
                    return bass.AP(
                        base.tensor,
                        offset,
                        [list(base.ap[0])] + [list(d) for d in dims],
                    )

                dcol = _win(dsh[:], 0, [(130, 3), (1, 3), (1, 128)])
                cent = _win(dsh[:], 131, [(0, 3), (0, 3), (1, 128)])
                rgb = _win(rg[:], 0, [(0, 9), (1, 128)])

                fA = mk.tile([128, 9 * 128], F32)
                fB = mk.tile([128, 9 * 128], F32)
                fC = mk.tile([128, 9 * 128], F32)
                nc.vector.tensor_tensor(fA[:], dcol, cent, ALU.subtract)   # et
                nc.vector.tensor_tensor(fB[:], fA[:], rgb, ALU.mult)       # tt
                nc.vector.tensor_scalar(fC[:], fB[:], -1.5, None, ALU.is_ge)  # u1
                nc.vector.scalar_tensor_tensor(
                    fA[:], fB[:], -0.5, fC[:], ALU.is_ge, ALU.add          # u2
                )
                nc.vector.scalar_tensor_tensor(
                    fC[:], fB[:], 0.5, fA[:], ALU.is_ge, ALU.add           # u3
                )
                bA = mk.tile([128, 9 * 128], BF16)
                bB = mk.tile([128, 9 * 128], BF16)
                bC = mk.tile([128, 9 * 128], BF16)
                bD = mk.tile([128, 9 * 128], BF16)
                nc.vector.scalar_tensor_tensor(
                    bA[:], fB[:], 1.5, fC[:], ALU.is_ge, ALU.add           # renc = r
                )
                # z = r-2; q = (z*z-4)*(z-c)
                nc.vector.tensor_scalar(bB[:], bA[:], -2.0, None, ALU.add)    # z
                nc.vector.tensor_tensor(bC[:], bB[:], bB[:], ALU.mult)        # z*z
                nc.vector.tensor_scalar(bA[:], bC[:], -4.0, None, ALU.add)    # g = z*z-4
                nc.vector.tensor_scalar(bD[:], bB[:], -QSEL_C, None, ALU.add) # z-c
                qc = mk.tile([128, 9 * 128], BF16)
                nc.vector.tensor_tensor(qc[:], bA[:], bD[:], ALU.mult)

                # store tap rows to DRAM bounce in pair order (row-major l),
                # one DMA per pair: dst walks (h, tap, w), src (h, [k1,k2], w)
                for p, (k1, k2) in enumerate(PAIRS):
                    dst = bass.AP(qd, 2 * p * L, [[128, 128], [L, 2], [1, 128]])
                    srcp = bass.AP(
                        qc.tensor,
                        k1 * 128,
                        [list(qc.ap[0]), [(k2 - k1) * 128, 2], [1, 128]],
                    )
                    eng = nc.sync if p % 2 == 0 else nc.gpsimd
                    eng.dma_start(dst, srcp)

            # ---- software pipeline over supers ----
            def casts(s):
                for ci in (2 * s, 2 * s + 1):
                    xs = stage.tile([C, CH], F32, tag="xs")
                    nc.sync.dma_start(xs[:], x_d[:, ci * CH : (ci + 1) * CH])
                    span = slice(BASE + ci * CH, BASE + (ci + 1) * CH)
                    nc.scalar.activation(tA[0:64, span], xs[:], AF.Copy)
                    nc.scalar.activation(tiles[1][0:64, span], xs[:], AF.Copy)
                    nc.scalar.activation(tiles[2][0:64, span], xs[:], AF.Copy)

            def shifts(s):
                a = BASE + s * SUP
                for t, row, sh in (
                    (tiles[0], 64, 1),
                    (tiles[1], 64, 128),
                    (tiles[2], 64, 130),
                ):
                    nc.gpsimd.dma_start(
                        t[row : row + 64, a : a + SUP],
                        tA[0:64, a + sh : a + sh + SUP],
                    )

            qtiles = {}

            def qload(s):
                s0 = s * SUP
                qtiles[s] = []
                for p in range(4):
                    q = qp.tile([128, SUP], BF16, tag=f"q{p}", name=f"q{p}_{s}")
                    nc.sync.dma_start(
                        q[:], bass.AP(qd, 2 * p * L + s0, [[L, 2], [0, 64], [1, SUP]])
                    )
                    qtiles[s].append(q)

            def compute(s):
                s0 = s * SUP
                qs = qtiles.pop(s)
                acc = psp.tile([128, CH], F32, tag="acc", name=f"acc{s}")

                def mm(gidx, rhs_fn, contract, start, stop):
                    for u in range(SUP // CH):
                        for h in range(2):
                            out_ap = acc[u * 64 : u * 64 + 64, h * 512 : (h + 1) * 512]
                            nc.tensor.matmul(
                                out_ap,
                                wp[0:contract, gidx * 64 : gidx * 64 + 64],
                                rhs_fn(u * CH + h * 512),
                                start=start,
                                stop=stop,
                            )

                # center tap: unmasked, contract 64
                mm(0, lambda c: tA[0:64, BASE + s0 + c : BASE + s0 + c + 512], 64,
                   True, False)
                for p in range(4):
                    src = tiles[PAIR_TILE[p]]
                    w0 = BASE + s0 + WINO[p]
                    y_prev = None
                    for j in (1, 2, 3):
                        y = yp.tile([128, SUP], BF16, tag=f"y{j}", name=f"y{j}_{s}_{p}")
                        in1 = src[:, w0 : w0 + SUP] if j == 1 else y_prev[:]
                        eng = nc.gpsimd if (p == 0 and j == 1) else nc.vector
                        eng.tensor_tensor(y[:], qs[p][:], in1, ALU.mult)
                        gidx = 1 + p * 3 + (j - 1)
                        mm(gidx, lambda c, yy=y: yy[:, c : c + 512], 128,
                           False, (p == 3 and j == 3))
                        y_prev = y

                for u in range(SUP // CH):
                    c = (SUP // CH) * s + u
                    osb = outp.tile([O, CH], F32, tag="osb", name=f"osb{c}")
                    nc.scalar.activation(osb[:], acc[u * 64 : u * 64 + 64, :], AF.Copy)
                    nc.sync.dma_start(out_d[:, c * CH : (c + 1) * CH], osb[:])

            for s in range(NSUP):
                casts(s)
                if s >= 1:
                    qload(s - 1)
                    shifts(s - 1)
                if s >= 2:
                    compute(s - 2)
            qload(NSUP - 1)
            shifts(NSUP - 1)
            compute(NSUP - 2)
            compute(NSUP - 1)

    noop_cls = type(nc.sync.nop().ins)
    _split_excess_waits(nc, noop_cls, max_waits=1)
    return nc


def _prep_weights(w0, w1, w2):
    ws = [w0, w1, w2]
    qv = {1: -2.25, 0: 1.0, -1: 3.75}
    M = np.array([[qv[z] ** j for j in (1, 2, 3)] for z in (1, 0, -1)])
    a = {z: np.linalg.solve(M, np.eye(3)[i]) for i, z in enumerate((1, 0, -1))}
    wt = [
        sum(a[z][j - 1] * ws[1 - z] for z in (1, 0, -1)).reshape(O, C, 9)
        for j in (1, 2, 3)
    ]
    wp = np.zeros((128, 13 * 64), dtype=np.float32)
    wp[0:64, 0:64] = ws[1].reshape(O, C, 9)[:, :, 4].T  # center
    for p, (k1, k2) in enumerate(PAIRS):
        for j in (1, 2, 3):
            gidx = 1 + p * 3 + (j - 1)
            wp[0:64, gidx * 64 : (gidx + 1) * 64] = wt[j - 1][:, :, k1].T
            wp[64:128, gidx * 64 : (gidx + 1) * 64] = wt[j - 1][:, :, k2].T
    return wp.astype(ml_dtypes.bfloat16)


_CACHE = {}


def kernel(x, depth, fx, weight_0, weight_1, weight_2, _trace=False):
    x = np.asarray(x, dtype=np.float32)
    depth = np.asarray(depth, dtype=np.float32)
    fx = np.asarray(fx, dtype=np.float32)
    wp = _prep_weights(
        np.asarray(weight_0, np.float32),
        np.asarray(weight_1, np.float32),
        np.asarray(weight_2, np.float32),
    )
    in_maps = []
    for i in range(N_IMG):
        in_maps.append(
            {
                "x": np.ascontiguousarray(x[i].reshape(C, L)),
                "depth": np.ascontiguousarray(depth[i, 0]),
                "rfx": np.full((128, 1), 1.0 / fx[i], dtype=np.float32),
                "wp": wp,
            }
        )
    nc = _build_graph()
    res = run_bass_kernel_spmd(nc, in_maps, core_ids=list(range(N_IMG)), trace=_trace)
    out = np.stack([res.results[i]["out"].reshape(O, H, W) for i in range(N_IMG)])
    if _trace:
        return out.astype(np.float32), res
    return out.astype(np.float32)


if __name__ == "__main__":
    rng = np.random.default_rng(0)
    ins = {
        "x": rng.standard_normal((N_IMG, C, H, W), dtype=np.float32),
        "depth": (1.0 + 9.0 * rng.random((N_IMG, 1, H, W))).astype(np.float32),
        "fx": (400.0 + 200.0 * rng.random(N_IMG)).astype(np.float32),
        "weight_0": rng.standard_normal((O, C, 3, 3), dtype=np.float32) * 0.04,
        "weight_1": rng.standard_normal((O, C, 3, 3), dtype=np.float32) * 0.04,
        "weight_2": rng.standard_normal((O, C, 3, 3), dtype=np.float32) * 0.04,
    }
    out = kernel(**ins)
    print("ran ok", out.shape, out.dtype)


# revision 11
# speedup vs baseline: 1.1072x; 1.1072x over previous
"""Depth-masked 3-branch 3x3 conv (Conv2.5D) on 8 TRN2 NeuronCores.

Data-parallel over N=8 images (1 image/core). Per core, a polynomial
selector replaces per-branch masking: with z = r - 2 (r = nearest-int
bin of (d_col-center)/grid in 0..4), the field q = (z^2-4)(z-1/4) is
zero for inactive taps and takes distinct values {3.75, 1, -2.25} for
branches 0/1/2. The three masked GEMM inputs per tap-pair are the chain
  y1 = Q*x, y2 = Q*y1, y3 = Q*y2      (DVE tensor_tensor, 2x mode)
with host-remixed weights Wt_j = sum_b a_j(b) W_b. The center tap is
always branch-1 active and is fed unmasked. Q is replicated across the
64 channel partitions by DMA broadcast (stride-0 free dim) from a DRAM
bounce buffer, so no DVE shuffles and no per-branch compare ops remain.
"""

import sys

sys.path.insert(0, "/opt/trn_rl_repo")

import numpy as np
import ml_dtypes

import concourse.bass as bass
import concourse.mybir as mybir
from concourse.bass_utils import run_bass_kernel_spmd
from concourse import tile
from concourse.vector_clock import VectorClock, ScopedClock

F32 = mybir.dt.float32
BF16 = mybir.dt.bfloat16
AF = mybir.ActivationFunctionType
ALU = mybir.AluOpType

N_IMG, C, O, H, W = 8, 64, 64, 128, 128
L = H * W
BASE = 256  # pad on each side of the x tiles (reads span +-259)
XW = BASE + L + BASE
CH = 1024  # psum / output chunk
SUP = 2048  # mult / broadcast super-chunk
NSUP = L // SUP
NCH = L // CH
# tap k = 3*(dh+1)+(dw+1); flat pixel offset dh*W+dw
OFF = [(k // 3 - 1) * W + (k % 3 - 1) for k in range(9)]
# non-center taps grouped in pairs sharing one physical x tile
PAIRS = [(0, 1), (6, 7), (2, 5), (3, 8)]
# tile shift of rows 64:128 relative to rows 0:64, per pair tile
PAIR_TILE = [0, 0, 1, 2]  # index into (tA, tB, tC)
TILE_SHIFT = [1, 128, 130]
WINO = [OFF[k1] for k1, _ in PAIRS]  # window offset per pair
QSEL_C = 0.25


def _patched_drain_and_barrier(self, tick_clock, wait_clock):
    # stock version puts every live sem wait on one drain -> walrus
    # "Too many sync wait commands"; emit one single-wait NOP per sem.
    ticks = list(tick_clock.global_clock)
    n = len(ticks)
    for i, t in enumerate(ticks):
        if t > 0:
            vec = [0] * n
            vec[i] = t
            nop = self.nc.sync.nop()
            wait_clock.add_sem_waits(nop.ins, ScopedClock({None: VectorClock(vec)}))
    self.nc.sync.drain()
    self.nc.all_engine_barrier()
    popped = self.nc._tile_sem_poison_stack.pop()
    assert popped is self._sem_poison
    self.nc.clear_and_free_semaphores(list(self.sems.allocated().values()))
    self.nc.all_engine_barrier()


tile.TileContext._drain_and_barrier = _patched_drain_and_barrier


def _split_excess_waits(nc, noop_cls, max_waits=1):
    # this walrus build rejects >1 sync-wait on several instruction
    # structs; hoist extras onto same-engine NoOps placed just before.
    for fn in nc.m.functions:
        for blk in fn.blocks:
            idx = 0
            while idx < len(blk.instructions):
                inst = blk.instructions[idx]
                si = inst.sync_info
                if si is not None and len(si.on_wait) > max_waits:
                    waits = list(si.on_wait)
                    si.on_wait = waits[-max_waits:]
                    pos = idx
                    for w in waits[:-max_waits]:
                        nop = noop_cls(
                            name=nc.get_next_instruction_name(), ins=[], outs=[]
                        )
                        nop.engine = inst.engine
                        nop.sync_info = mybir.SyncInfo(on_wait=[w], on_update=[])
                        nc.register_instruction(nop)
                        blk.instructions.insert(pos, nop)
                        pos += 1
                        idx += 1
                idx += 1


def _build_graph():
    nc = bass.Bass()
    x_d = nc.declare_dram_parameter("x", [C, L], F32, isOutput=False)
    dep_d = nc.declare_dram_parameter("depth", [H, W], F32, isOutput=False)
    rfx_d = nc.declare_dram_parameter("rfx", [128, 1], F32, isOutput=False)
    wp_d = nc.declare_dram_parameter("wp", [128, 13 * 64], BF16, isOutput=False)
    out_d = nc.declare_dram_parameter("out", [O, L], F32, isOutput=True)
    qd = nc.dram_tensor("qd", (8, L), BF16, kind="Internal")

    with tile.TileContext(nc) as tc:
        with (
            tc.tile_pool(name="big", bufs=1) as big,
            tc.tile_pool(name="stage", bufs=3) as stage,
            tc.tile_pool(name="qp", bufs=1) as qp,
            tc.tile_pool(name="yp", bufs=2) as yp,
            tc.tile_pool(name="outp", bufs=3) as outp,
            tc.tile_pool(name="psum", bufs=4, space=bass.MemorySpace.PSUM) as psp,
        ):
            wp = big.tile([128, 13 * 64], BF16)
            nc.sync.dma_start(wp[:], wp_d[:])

            tiles = [big.tile([128, XW], BF16, name=f"t{i}") for i in range(3)]
            for t in tiles:
                nc.gpsimd.memset(t[:, 0:BASE], 0.0)
                nc.gpsimd.memset(t[:, BASE + L : XW], 0.0)
            tA = tiles[0]

            # ---- depth -> q selector field (128h x 9*128w) ----
            with tc.tile_pool(name="mk", bufs=1) as mk:
                dsh = mk.tile([128, 3 * 130], F32)
                nc.vector.memset(dsh[:], 0.0)
                nc.sync.dma_start(dsh[:, 131:259], dep_d[:, :])
                nc.sync.dma_start(dsh[0:127, 261:389], dep_d[1:128, :])
                nc.sync.dma_start(dsh[1:128, 1:129], dep_d[0:127, :])
                rfx = mk.tile([128, 1], F32)
                nc.sync.dma_start(rfx[:], rfx_d[:])

                g = mk.tile([128, 128], F32)
                nc.vector.tensor_scalar(g[:], dsh[:, 131:259], rfx[:], None, ALU.mult)
                rg = mk.tile([128, 128], F32)
                nc.vector.reciprocal(rg[:], g[:])

                def _win(base, offset, dims):
                    return bass.AP(
                        base.tensor,
                        offset,
                        [list(base.ap[0])] + [list(d) for d in dims],
                    )

                dcol = _win(dsh[:], 0, [(130, 3), (1, 3), (1, 128)])
                cent = _win(dsh[:], 131, [(0, 3), (0, 3), (1, 128)])
                rgb = _win(rg[:], 0, [(0, 9), (1, 128)])

                fA = mk.tile([128, 9 * 128], F32)
                fB = mk.tile([128, 9 * 128], F32)
                fC = mk.tile([128, 9 * 128], F32)
                nc.vector.tensor_tensor(fA[:], dcol, cent, ALU.subtract)   # et
                nc.vector.tensor_tensor(fB[:], fA[:], rgb, ALU.mult)       # tt
                nc.vector.tensor_scalar(fC[:], fB[:], -1.5, None, ALU.is_ge)  # u1
                nc.vector.scalar_tensor_tensor(
                    fA[:], fB[:], -0.5, fC[:], ALU.is_ge, ALU.add          # u2
                )
                nc.vector.scalar_tensor_tensor(
                    fC[:], fB[:], 0.5, fA[:], ALU.is_ge, ALU.add           # u3
                )
                bA = mk.tile([128, 9 * 128], BF16)
                bB = mk.tile([128, 9 * 128], BF16)
                bC = mk.tile([128, 9 * 128], BF16)
                bD = mk.tile([128, 9 * 128], BF16)
                nc.vector.scalar_tensor_tensor(
                    bA[:], fB[:], 1.5, fC[:], ALU.is_ge, ALU.add           # renc = r
                )
                # z = r-2; q = (z*z-4)*(z-c)
                nc.vector.tensor_scalar(bB[:], bA[:], -2.0, None, ALU.add)    # z
                nc.vector.tensor_tensor(bC[:], bB[:], bB[:], ALU.mult)        # z*z
                nc.vector.tensor_scalar(bA[:], bC[:], -4.0, None, ALU.add)    # g = z*z-4
                nc.vector.tensor_scalar(bD[:], bB[:], -QSEL_C, None, ALU.add) # z-c
                qc = mk.tile([128, 9 * 128], BF16)
                nc.vector.tensor_tensor(qc[:], bA[:], bD[:], ALU.mult)

                # store tap rows to DRAM bounce in pair order (row-major l),
                # one DMA per pair: dst walks (h, tap, w), src (h, [k1,k2], w)
                for p, (k1, k2) in enumerate(PAIRS):
                    dst = bass.AP(qd, 2 * p * L, [[128, 128], [L, 2], [1, 128]])
                    srcp = bass.AP(
                        qc.tensor,
                        k1 * 128,
                        [list(qc.ap[0]), [(k2 - k1) * 128, 2], [1, 128]],
                    )
                    eng = nc.sync if p % 2 == 0 else nc.gpsimd
                    eng.dma_start(dst, srcp)

            # ---- software pipeline over supers ----
            def casts(s):
                for ci in (2 * s, 2 * s + 1):
                    xs = stage.tile([C, CH], F32, tag="xs")
                    nc.sync.dma_start(xs[:], x_d[:, ci * CH : (ci + 1) * CH])
                    span = slice(BASE + ci * CH, BASE + (ci + 1) * CH)
                    nc.scalar.activation(tA[0:64, span], xs[:], AF.Copy)
                    nc.scalar.activation(tiles[1][0:64, span], xs[:], AF.Copy)
                    nc.scalar.activation(tiles[2][0:64, span], xs[:], AF.Copy)

            def shifts(s):
                a = BASE + s * SUP
                for t, row, sh in (
                    (tiles[0], 64, 1),
                    (tiles[1], 64, 128),
                    (tiles[2], 64, 130),
                ):
                    nc.gpsimd.dma_start(
                        t[row : row + 64, a : a + SUP],
                        tA[0:64, a + sh : a + sh + SUP],
                    )

            qtiles = {}

            def qload(s):
                s0 = s * SUP
                qtiles[s] = []
                for p in range(4):
                    q = qp.tile([128, SUP], BF16, tag=f"q{p}", name=f"q{p}_{s}")
                    nc.sync.dma_start(
                        q[:], bass.AP(qd, 2 * p * L + s0, [[L, 2], [0, 64], [1, SUP]])
                    )
                    qtiles[s].append(q)

            def compute(s):
                s0 = s * SUP
                qs = qtiles.pop(s)
                acc = psp.tile([128, CH], F32, tag="acc", name=f"acc{s}")

                def mm(gidx, rhs_fn, contract, start, stop):
                    for u in range(SUP // CH):
                        for h in range(2):
                            out_ap = acc[u * 64 : u * 64 + 64, h * 512 : (h + 1) * 512]
                            nc.tensor.matmul(
                                out_ap,
                                wp[0:contract, gidx * 64 : gidx * 64 + 64],
                                rhs_fn(u * CH + h * 512),
                                start=start,
                                stop=stop,
                            )

                # center tap: unmasked, contract 64
                mm(0, lambda c: tA[0:64, BASE + s0 + c : BASE + s0 + c + 512], 64,
                   True, False)
                for p in range(4):
                    src = tiles[PAIR_TILE[p]]
                    w0 = BASE + s0 + WINO[p]
                    y_prev = None
                    for j in (1, 2, 3):
                        y = yp.tile([128, SUP], BF16, tag=f"y{j}", name=f"y{j}_{s}_{p}")
                        in1 = src[:, w0 : w0 + SUP] if j == 1 else y_prev[:]
                        nc.vector.tensor_tensor(y[:], qs[p][:], in1, ALU.mult)
                        gidx = 1 + p * 3 + (j - 1)
                        mm(gidx, lambda c, yy=y: yy[:, c : c + 512], 128,
                           False, (p == 3 and j == 3))
                        y_prev = y

                for u in range(SUP // CH):
                    c = (SUP // CH) * s + u
                    osb = outp.tile([O, CH], F32, tag="osb", name=f"osb{c}")
                    nc.scalar.activation(osb[:], acc[u * 64 : u * 64 + 64, :], AF.Copy)
                    nc.sync.dma_start(out_d[:, c * CH : (c + 1) * CH], osb[:])

            for s in range(NSUP):
                casts(s)
                if s >= 1:
                    qload(s - 1)
                    shifts(s - 1)
                if s >= 2:
                    compute(s - 2)
            qload(NSUP - 1)
            shifts(NSUP - 1)
            compute(NSUP - 2)
            compute(NSUP - 1)

    noop_cls = type(nc.sync.nop().ins)
    _split_excess_waits(nc, noop_cls, max_waits=1)
    return nc


def _prep_weights(w0, w1, w2):
    ws = [w0, w1, w2]
    qv = {1: -2.25, 0: 1.0, -1: 3.75}
    M = np.array([[qv[z] ** j for j in (1, 2, 3)] for z in (1, 0, -1)])
    a = {z: np.linalg.solve(M, np.eye(3)[i]) for i, z in enumerate((1, 0, -1))}
    wt = [
        sum(a[z][j - 1] * ws[1 - z] for z in (1, 0, -1)).reshape(O, C, 9)
        for j in (1, 2, 3)
    ]
    wp = np.zeros((128, 13 * 64), dtype=np.float32)
    wp[0:64, 0:64] = ws[1].reshape(O, C, 9)[:, :, 4].T  # center
    for p, (k1, k2) in enumerate(PAIRS):
        for j in (1, 2, 3):
            gidx = 1 + p * 3 + (j - 1)
            wp[0:64, gidx * 64 : (gidx + 1) * 64] = wt[j - 1][:, :, k1].T
            wp[64:128, gidx * 64 : (gidx + 1) * 64] = wt[j - 1][:, :, k2].T
    return wp.astype(ml_dtypes.bfloat16)


_CACHE = {}


def kernel(x, depth, fx, weight_0, weight_1, weight_2, _trace=False):
    x = np.asarray(x, dtype=np.float32)
    depth = np.asarray(depth, dtype=np.float32)
    fx = np.asarray(fx, dtype=np.float32)
    wp = _prep_weights(
        np.asarray(weight_0, np.float32),
        np.asarray(weight_1, np.float32),
        np.asarray(weight_2, np.float32),
    )
    in_maps = []
    for i in range(N_IMG):
        in_maps.append(
            {
                "x": np.ascontiguousarray(x[i].reshape(C, L)),
                "depth": np.ascontiguousarray(depth[i, 0]),
                "rfx": np.full((128, 1), 1.0 / fx[i], dtype=np.float32),
                "wp": wp,
            }
        )
    nc = _build_graph()
    res = run_bass_kernel_spmd(nc, in_maps, core_ids=list(range(N_IMG)), trace=_trace)
    out = np.stack([res.results[i]["out"].reshape(O, H, W) for i in range(N_IMG)])
    if _trace:
        return out.astype(np.float32), res
    return out.astype(np.float32)


if __name__ == "__main__":
    rng = np.random.default_rng(0)
    ins = {
        "x": rng.standard_normal((N_IMG, C, H, W), dtype=np.float32),
        "depth": (1.0 + 9.0 * rng.random((N_IMG, 1, H, W))).astype(np.float32),
        "fx": (400.0 + 200.0 * rng.random(N_IMG)).astype(np.float32),
        "weight_0": rng.standard_normal((O, C, 3, 3), dtype=np.float32) * 0.04,
        "weight_1": rng.standard_normal((O, C, 3, 3), dtype=np.float32) * 0.04,
        "weight_2": rng.standard_normal((O, C, 3, 3), dtype=np.float32) * 0.04,
    }
    out = kernel(**ins)
    print("ran ok", out.shape, out.dtype)


# revision 13
# speedup vs baseline: 1.1119x; 1.0043x over previous
"""Depth-masked 3-branch 3x3 conv (Conv2.5D) on 8 TRN2 NeuronCores.

Data-parallel over N=8 images (1 image/core). Per core, a polynomial
selector replaces per-branch masking: with z = r - 2 (r = nearest-int
bin of (d_col-center)/grid in 0..4), the field q = (z^2-4)(z-1/4) is
zero for inactive taps and takes distinct values {3.75, 1, -2.25} for
branches 0/1/2. The three masked GEMM inputs per tap-pair are the chain
  y1 = Q*x, y2 = Q*y1, y3 = Q*y2      (DVE tensor_tensor, 2x mode)
with host-remixed weights Wt_j = sum_b a_j(b) W_b. The center tap is
always branch-1 active and is fed unmasked. Q is replicated across the
64 channel partitions by DMA broadcast (stride-0 free dim) from a DRAM
bounce buffer, so no DVE shuffles and no per-branch compare ops remain.
"""

import sys

sys.path.insert(0, "/opt/trn_rl_repo")

import numpy as np
import ml_dtypes

import concourse.bass as bass
import concourse.mybir as mybir
from concourse.bass_utils import run_bass_kernel_spmd
from concourse import tile
from concourse.vector_clock import VectorClock, ScopedClock

F32 = mybir.dt.float32
BF16 = mybir.dt.bfloat16
AF = mybir.ActivationFunctionType
ALU = mybir.AluOpType

N_IMG, C, O, H, W = 8, 64, 64, 128, 128
L = H * W
BASE = 256  # pad on each side of the x tiles (reads span +-259)
XW = BASE + L + BASE
CH = 1024  # psum / output chunk
SUP = 2048  # mult / broadcast super-chunk
NSUP = L // SUP
NCH = L // CH
# tap k = 3*(dh+1)+(dw+1); flat pixel offset dh*W+dw
OFF = [(k // 3 - 1) * W + (k % 3 - 1) for k in range(9)]
# non-center taps grouped in pairs sharing one physical x tile
PAIRS = [(0, 1), (6, 7), (2, 5), (3, 8)]
# tile shift of rows 64:128 relative to rows 0:64, per pair tile
PAIR_TILE = [0, 0, 1, 2]  # index into (tA, tB, tC)
TILE_SHIFT = [1, 128, 130]
WINO = [OFF[k1] for k1, _ in PAIRS]  # window offset per pair
QSEL_C = 0.25


def _patched_drain_and_barrier(self, tick_clock, wait_clock):
    # stock version puts every live sem wait on one drain -> walrus
    # "Too many sync wait commands"; emit one single-wait NOP per sem.
    ticks = list(tick_clock.global_clock)
    n = len(ticks)
    for i, t in enumerate(ticks):
        if t > 0:
            vec = [0] * n
            vec[i] = t
            nop = self.nc.sync.nop()
            wait_clock.add_sem_waits(nop.ins, ScopedClock({None: VectorClock(vec)}))
    self.nc.sync.drain()
    self.nc.all_engine_barrier()
    popped = self.nc._tile_sem_poison_stack.pop()
    assert popped is self._sem_poison
    self.nc.clear_and_free_semaphores(list(self.sems.allocated().values()))
    self.nc.all_engine_barrier()


tile.TileContext._drain_and_barrier = _patched_drain_and_barrier


def _split_excess_waits(nc, noop_cls, max_waits=1):
    # this walrus build rejects >1 sync-wait on several instruction
    # structs; hoist extras onto same-engine NoOps placed just before.
    for fn in nc.m.functions:
        for blk in fn.blocks:
            idx = 0
            while idx < len(blk.instructions):
                inst = blk.instructions[idx]
                si = inst.sync_info
                if si is not None and len(si.on_wait) > max_waits:
                    waits = list(si.on_wait)
                    si.on_wait = waits[-max_waits:]
                    pos = idx
                    for w in waits[:-max_waits]:
                        nop = noop_cls(
                            name=nc.get_next_instruction_name(), ins=[], outs=[]
                        )
                        nop.engine = inst.engine
                        nop.sync_info = mybir.SyncInfo(on_wait=[w], on_update=[])
                        nc.register_instruction(nop)
                        blk.instructions.insert(pos, nop)
                        pos += 1
                        idx += 1
                idx += 1


def _build_graph():
    nc = bass.Bass()
    x_d = nc.declare_dram_parameter("x", [C, L], F32, isOutput=False)
    dep_d = nc.declare_dram_parameter("depth", [H, W], F32, isOutput=False)
    rfx_d = nc.declare_dram_parameter("rfx", [128, 1], F32, isOutput=False)
    wp_d = nc.declare_dram_parameter("wp", [128, 13 * 64], BF16, isOutput=False)
    out_d = nc.declare_dram_parameter("out", [O, L], F32, isOutput=True)
    qd = nc.dram_tensor("qd", (8, L), BF16, kind="Internal")

    with tile.TileContext(nc) as tc:
        with (
            tc.tile_pool(name="big", bufs=1) as big,
            tc.tile_pool(name="stage", bufs=3) as stage,
            tc.tile_pool(name="qp", bufs=1) as qp,
            tc.tile_pool(name="yp", bufs=2) as yp,
            tc.tile_pool(name="outp", bufs=3) as outp,
            tc.tile_pool(name="psum", bufs=4, space=bass.MemorySpace.PSUM) as psp,
        ):
            wp = big.tile([128, 13 * 64], BF16)
            nc.sync.dma_start(wp[:], wp_d[:])

            tiles = [big.tile([128, XW], BF16, name=f"t{i}") for i in range(3)]
            for t in tiles:
                nc.gpsimd.memset(t[:, 0:BASE], 0.0)
                nc.gpsimd.memset(t[:, BASE + L : XW], 0.0)
            tA = tiles[0]

            # ---- depth -> q selector field (128h x 9*128w) ----
            with tc.tile_pool(name="mk", bufs=1) as mk:
                dsh = mk.tile([128, 3 * 130], F32)
                nc.vector.memset(dsh[:], 0.0)
                nc.sync.dma_start(dsh[:, 131:259], dep_d[:, :])
                nc.sync.dma_start(dsh[0:127, 261:389], dep_d[1:128, :])
                nc.sync.dma_start(dsh[1:128, 1:129], dep_d[0:127, :])
                rfx = mk.tile([128, 1], F32)
                nc.sync.dma_start(rfx[:], rfx_d[:])

                g = mk.tile([128, 128], F32)
                nc.vector.tensor_scalar(g[:], dsh[:, 131:259], rfx[:], None, ALU.mult)
                rg = mk.tile([128, 128], F32)
                nc.vector.reciprocal(rg[:], g[:])

                def _win(base, offset, dims):
                    return bass.AP(
                        base.tensor,
                        offset,
                        [list(base.ap[0])] + [list(d) for d in dims],
                    )

                dcol = _win(dsh[:], 0, [(130, 3), (1, 3), (1, 128)])
                cent = _win(dsh[:], 131, [(0, 3), (0, 3), (1, 128)])
                rgb = _win(rg[:], 0, [(0, 9), (1, 128)])

                fA = mk.tile([128, 9 * 128], F32)
                fB = mk.tile([128, 9 * 128], F32)
                fC = mk.tile([128, 9 * 128], F32)
                nc.vector.tensor_tensor(fA[:], dcol, cent, ALU.subtract)   # et
                nc.vector.tensor_tensor(fB[:], fA[:], rgb, ALU.mult)       # tt
                nc.vector.tensor_scalar(fC[:], fB[:], -1.5, None, ALU.is_ge)  # u1
                nc.vector.scalar_tensor_tensor(
                    fA[:], fB[:], -0.5, fC[:], ALU.is_ge, ALU.add          # u2
                )
                nc.vector.scalar_tensor_tensor(
                    fC[:], fB[:], 0.5, fA[:], ALU.is_ge, ALU.add           # u3
                )
                bA = mk.tile([128, 9 * 128], BF16)
                bB = mk.tile([128, 9 * 128], BF16)
                bC = mk.tile([128, 9 * 128], BF16)
                bD = mk.tile([128, 9 * 128], BF16)
                nc.vector.scalar_tensor_tensor(
                    bA[:], fB[:], 1.5, fC[:], ALU.is_ge, ALU.add           # renc = r
                )
                # z = r-2; q = (z*z-4)*(z-c)
                nc.vector.tensor_scalar(bB[:], bA[:], -2.0, None, ALU.add)    # z
                nc.vector.tensor_tensor(bC[:], bB[:], bB[:], ALU.mult)        # z*z
                nc.vector.tensor_scalar(bA[:], bC[:], -4.0, None, ALU.add)    # g = z*z-4
                nc.vector.tensor_scalar(bD[:], bB[:], -QSEL_C, None, ALU.add) # z-c
                qc = bC  # zz is dead; reuse its buffer for q
                nc.vector.tensor_tensor(qc[:], bA[:], bD[:], ALU.mult)

                # store tap rows to DRAM bounce in pair order (row-major l),
                # one DMA per pair: dst walks (h, tap, w), src (h, [k1,k2], w)
                for p, (k1, k2) in enumerate(PAIRS):
                    dst = bass.AP(qd, 2 * p * L, [[128, 128], [L, 2], [1, 128]])
                    srcp = bass.AP(
                        qc.tensor,
                        k1 * 128,
                        [list(qc.ap[0]), [(k2 - k1) * 128, 2], [1, 128]],
                    )
                    eng = nc.sync if p % 2 == 0 else nc.gpsimd
                    eng.dma_start(dst, srcp)

            # ---- software pipeline over supers ----
            def casts(s):
                for ci in (2 * s, 2 * s + 1):
                    xs = stage.tile([C, CH], F32, tag="xs")
                    nc.sync.dma_start(xs[:], x_d[:, ci * CH : (ci + 1) * CH])
                    span = slice(BASE + ci * CH, BASE + (ci + 1) * CH)
                    nc.scalar.activation(tA[0:64, span], xs[:], AF.Copy)
                    nc.scalar.activation(tiles[1][0:64, span], xs[:], AF.Copy)
                    nc.scalar.activation(tiles[2][0:64, span], xs[:], AF.Copy)

            def shifts(s):
                a = BASE + s * SUP
                for t, row, sh in (
                    (tiles[0], 64, 1),
                    (tiles[1], 64, 128),
                    (tiles[2], 64, 130),
                ):
                    nc.gpsimd.dma_start(
                        t[row : row + 64, a : a + SUP],
                        tA[0:64, a + sh : a + sh + SUP],
                    )

            qtiles = {}

            def qload(s):
                s0 = s * SUP
                qtiles[s] = []
                for p in range(4):
                    q = qp.tile([128, SUP], BF16, tag=f"q{p}", name=f"q{p}_{s}")
                    nc.sync.dma_start(
                        q[:], bass.AP(qd, 2 * p * L + s0, [[L, 2], [0, 64], [1, SUP]])
                    )
                    qtiles[s].append(q)

            def compute(s):
                s0 = s * SUP
                qs = qtiles.pop(s)
                acc = psp.tile([128, CH], F32, tag="acc", name=f"acc{s}")

                def mm(gidx, rhs_fn, contract, start, stop):
                    for u in range(SUP // CH):
                        for h in range(2):
                            out_ap = acc[u * 64 : u * 64 + 64, h * 512 : (h + 1) * 512]
                            nc.tensor.matmul(
                                out_ap,
                                wp[0:contract, gidx * 64 : gidx * 64 + 64],
                                rhs_fn(u * CH + h * 512),
                                start=start,
                                stop=stop,
                            )

                # Pool-prefetched y1 for pairs 1 and 2 (slow engine, hidden
                # behind DVE work on pairs 0 and 3)
                POOL_Y1 = (1, 2)
                y1_pool = {}
                for p in POOL_Y1:
                    src = tiles[PAIR_TILE[p]]
                    w0 = BASE + s0 + WINO[p]
                    y = yp.tile([128, SUP], BF16, tag=f"y1p{p}", name=f"y1_{s}_{p}")
                    nc.gpsimd.tensor_tensor(y[:], qs[p][:], src[:, w0 : w0 + SUP],
                                            ALU.mult)
                    y1_pool[p] = y
                # center tap: unmasked, contract 64
                mm(0, lambda c: tA[0:64, BASE + s0 + c : BASE + s0 + c + 512], 64,
                   True, False)
                for p in (0, 3, 1, 2):
                    src = tiles[PAIR_TILE[p]]
                    w0 = BASE + s0 + WINO[p]
                    y_prev = None
                    for j in (1, 2, 3):
                        if j == 1 and p in POOL_Y1:
                            y = y1_pool[p]
                        else:
                            y = yp.tile([128, SUP], BF16, tag=f"y{j}",
                                        name=f"y{j}_{s}_{p}")
                            in1 = src[:, w0 : w0 + SUP] if j == 1 else y_prev[:]
                            nc.vector.tensor_tensor(y[:], qs[p][:], in1, ALU.mult)
                        gidx = 1 + p * 3 + (j - 1)
                        mm(gidx, lambda c, yy=y: yy[:, c : c + 512], 128,
                           False, (p == 2 and j == 3))
                        y_prev = y

                for u in range(SUP // CH):
                    c = (SUP // CH) * s + u
                    osb = outp.tile([O, CH], F32, tag="osb", name=f"osb{c}")
                    nc.scalar.activation(osb[:], acc[u * 64 : u * 64 + 64, :], AF.Copy)
                    nc.sync.dma_start(out_d[:, c * CH : (c + 1) * CH], osb[:])

            for s in range(NSUP):
                casts(s)
                if s >= 1:
                    qload(s - 1)
                    shifts(s - 1)
                if s >= 2:
                    compute(s - 2)
            qload(NSUP - 1)
            shifts(NSUP - 1)
            compute(NSUP - 2)
            compute(NSUP - 1)

    noop_cls = type(nc.sync.nop().ins)
    _split_excess_waits(nc, noop_cls, max_waits=1)
    return nc


def _prep_weights(w0, w1, w2):
    ws = [w0, w1, w2]
    qv = {1: -2.25, 0: 1.0, -1: 3.75}
    M = np.array([[qv[z] ** j for j in (1, 2, 3)] for z in (1, 0, -1)])
    a = {z: np.linalg.solve(M, np.eye(3)[i]) for i, z in enumerate((1, 0, -1))}
    wt = [
        sum(a[z][j - 1] * ws[1 - z] for z in (1, 0, -1)).reshape(O, C, 9)
        for j in (1, 2, 3)
    ]
    wp = np.zeros((128, 13 * 64), dtype=np.float32)
    wp[0:64, 0:64] = ws[1].reshape(O, C, 9)[:, :, 4].T  # center
    for p, (k1, k2) in enumerate(PAIRS):
        for j in (1, 2, 3):
            gidx = 1 + p * 3 + (j - 1)
            wp[0:64, gidx * 64 : (gidx + 1) * 64] = wt[j - 1][:, :, k1].T
            wp[64:128, gidx * 64 : (gidx + 1) * 64] = wt[j - 1][:, :, k2].T
    return wp.astype(ml_dtypes.bfloat16)


_CACHE = {}


def kernel(x, depth, fx, weight_0, weight_1, weight_2, _trace=False):
    x = np.asarray(x, dtype=np.float32)
    depth = np.asarray(depth, dtype=np.float32)
    fx = np.asarray(fx, dtype=np.float32)
    wp = _prep_weights(
        np.asarray(weight_0, np.float32),
        np.asarray(weight_1, np.float32),
        np.asarray(weight_2, np.float32),
    )
    in_maps = []
    for i in range(N_IMG):
        in_maps.append(
            {
                "x": np.ascontiguousarray(x[i].reshape(C, L)),
                "depth": np.ascontiguousarray(depth[i, 0]),
                "rfx": np.full((128, 1), 1.0 / fx[i], dtype=np.float32),
                "wp": wp,
            }
        )
    nc = _build_graph()
    res = run_bass_kernel_spmd(nc, in_maps, core_ids=list(range(N_IMG)), trace=_trace)
    out = np.stack([res.results[i]["out"].reshape(O, H, W) for i in range(N_IMG)])
    if _trace:
        return out.astype(np.float32), res
    return out.astype(np.float32)


if __name__ == "__main__":
    rng = np.random.default_rng(0)
    ins = {
        "x": rng.standard_normal((N_IMG, C, H, W), dtype=np.float32),
        "depth": (1.0 + 9.0 * rng.random((N_IMG, 1, H, W))).astype(np.float32),
        "fx": (400.0 + 200.0 * rng.random(N_IMG)).astype(np.float32),
        "weight_0": rng.standard_normal((O, C, 3, 3), dtype=np.float32) * 0.04,
        "weight_1": rng.standard_normal((O, C, 3, 3), dtype=np.float32) * 0.04,
        "weight_2": rng.standard_normal((O, C, 3, 3), dtype=np.float32) * 0.04,
    }
    out = kernel(**ins)
    print("ran ok", out.shape, out.dtype)
